# revision 10
# baseline (speedup 1.0000x reference)
"""Trainium2 Bass kernel for nn_Conv2DLayer_16011638080159.

Math: out = C * (x @ weight.sum(0))   with x [524288, 512], weight [9, 512].
A row-wise dot product of x with w_eff = C * weight.sum(0).

Strategy (v5, PE-based, fp16 traffic, tapered tail):
  - Pure data parallel: shard x rows across 8 NeuronCores (65536 rows each).
  - The kernel is HBM-DMA bound, so HBM traffic is halved by casting x to
    fp16 on the host (l2 rel err 2.5e-4, gate is 2e-2). The host also
    pre-transposes each shard to x^T [512, 65536] so the TensorEngine can
    reduce along the channel (partition) axis with plain contiguous DMAs.
  - Per core: stream x^T tiles [128, rb] fp16 (4 channel chunks per block)
    on the SP HWDGE ring. For each group of 512 rows: 4 PSUM-accumulating
    matmuls, stationary = w chunk [128, 1], moving = x^T slice [128, 512],
    out = psum [1, 512]. PSUM rows are copied (cast) to fp16 slabs
    alternating DVE/ACT; one out-DMA per block on the ACT ring.
  - Block sizes taper at the end (7x8192 then 4096/2048/1024/1024): the
    final block's matmuls are the only PE work serialized after the last
    HBM byte, so a small last block cuts ~10 us of tail. All tiles are
    allocated at the full 8192 size (fixed pool shapes); small blocks use
    a prefix.
  - w is scaled by 1/16 so fp16 partials stay in range; host multiplies
    the gathered output by 16 in fp32.
  - Engine budgets/core: DMA ~188 us (bound), PE ~132 us, DVE/ACT ~50 us.
"""

import numpy as np

import concourse.bacc as bacc
import concourse.bass as bass
import concourse.tile as tile
from concourse import mybir
from concourse.bass_utils import run_bass_kernel_spmd

B = 524288         # total rows
C = 512            # row length (channels)
N_CORES = 8
BS = B // N_CORES  # 65536 rows per core
P = 128            # SBUF/PSUM partitions
KC = C // P        # 4 channel chunks of 128
RB = 8192          # full super-block rows (also the tile allocation size)
MB = 2048
BLOCKS = [RB] * 7 + [MB] * 4              # sums to BS
OUT_SCALE = 16.0   # host multiplies fp16 device output by this (w /= 16)

assert sum(BLOCKS) == BS

_NC_CACHE = None
LAST_RESULT = None  # BassKernelResults of the most recent run (for profiling)


def _build() -> bass.Bass:
    nc = bacc.Bacc(None, target_bir_lowering=False, debug=False)
    xT = nc.dram_tensor("xT", [C, BS], mybir.dt.float16, kind="ExternalInput")
    w = nc.dram_tensor("w", [P, KC], mybir.dt.float16, kind="ExternalInput")
    out = nc.dram_tensor("out", [BS], mybir.dt.float16, kind="ExternalOutput")

    xc = xT.rearrange("(k p) f -> k p f", k=KC, p=P)   # chunk k -> [128, BS]
    ov = out.rearrange("(o f) -> o f", o=1)

    with tile.TileContext(nc) as tc:
        with (
            tc.tile_pool(name="const", bufs=1) as cpool,
            tc.tile_pool(name="xs", bufs=2) as xs,
            tc.tile_pool(name="xm", bufs=2) as xm,
            tc.psum_pool(name="ps", bufs=4) as ps,
            tc.tile_pool(name="res", bufs=2) as res,
            tc.tile_pool(name="rm", bufs=2) as rm,
        ):
            w_t = cpool.tile([P, KC], mybir.dt.float16)
            nc.sync.dma_start(out=w_t[:], in_=w[:, :])
            r0 = 0
            for t, rb in enumerate(BLOCKS):
                xk = []
                for k in range(KC):
                    if rb == RB:
                        x_t = xs.tile([P, RB], mybir.dt.float16, tag=f"x{k}")
                    else:
                        x_t = xm.tile([P, MB], mybir.dt.float16, tag=f"xm{k}")
                    nc.sync.dma_start(out=x_t[:], in_=xc[k][:, r0:r0 + rb])
                    xk.append(x_t)
                if rb == RB:
                    o_t = res.tile([1, RB], mybir.dt.float16)
                else:
                    o_t = rm.tile([1, MB], mybir.dt.float16, tag="om")
                for j in range(rb // 512):
                    g = r0 // 512 + j
                    p_t = ps.tile([1, 512], mybir.dt.float32)
                    for k in range(KC):
                        nc.tensor.matmul(
                            p_t[:],
                            lhsT=w_t[:, k:k + 1],
                            rhs=xk[k][:, j * 512:(j + 1) * 512],
                            start=(k == 0),
                            stop=(k == KC - 1),
                        )
                    dst = o_t[:, j * 512:(j + 1) * 512]
                    if g % 2 == 0:
                        nc.vector.tensor_copy(dst, p_t[:])
                    else:
                        nc.scalar.copy(dst, p_t[:])
                # off the SP ring so x-tile DMA issue is never delayed
                nc.scalar.dma_start(out=ov[:, r0:r0 + rb], in_=o_t[:])
                r0 += rb
    nc.finalize()
    return nc


def kernel(x: np.ndarray, weight: np.ndarray) -> np.ndarray:
    global _NC_CACHE, LAST_RESULT
    x = np.asarray(x)
    weight = np.asarray(weight, dtype=np.float32)

    w_eff = (C / OUT_SCALE * weight.sum(axis=0)).astype(np.float16)  # [C]
    w_stat = np.ascontiguousarray(w_eff.reshape(KC, P).T)            # [P, KC]

    # fp16 cast (contiguous pass), then per-shard transpose to [C, BS]
    x16 = np.asarray(x, dtype=np.float16)
    shards = [
        np.ascontiguousarray(x16[i * BS:(i + 1) * BS].T) for i in range(N_CORES)
    ]

    if _NC_CACHE is None:
        _NC_CACHE = _build()

    in_maps = [{"xT": shards[i], "w": w_stat} for i in range(N_CORES)]
    LAST_RESULT = run_bass_kernel_spmd(
        _NC_CACHE, in_maps, core_ids=list(range(N_CORES))
    )
    return np.concatenate(
        [r["out"].astype(np.float32) * OUT_SCALE for r in LAST_RESULT.results]
    )
